# revision 15
# baseline (speedup 1.0000x reference)
"""ChebConv (K=3) GNN message-passing kernel for 8 Trainium2 NeuronCores.

Strategy (node sharding, per sharding hint):
 - 50000 nodes split into 8 contiguous shards of 6250, padded to 6272 = 49*128.
 - Within each shard nodes are ordered by (integer) degree so each 128-row
   tile has near-uniform max degree -> low padding in the slot layout.
 - Edge (r, c) lives on the core owning r, at slot (tile(r), part(r), j).
 - The symmetric normalization is folded on the HOST into the per-edge
   weights:  w_e = -dis[row]*ew_e*dis[col]  (L_hat = -D^-1/2 A D^-1/2), so
   the device never computes deg/dis, and pass 1 gathers the replicated raw
   x table directly (no first AllGather):
     T1   = segsum_j(w * gather(x_rep, col))        per local (tile,node)
     T1  -> AllGather -> T1_full (bf16)
     P2   = segsum_j(w * gather(T1_full, col))
     out  = relu(T0@(W0-W2) + T1@W1 + P2@(2*W2) + b)
   (T2 = 2*P2 - T0 is folded into the weight matrices on the host.)

 Gathers move 256B bf16 node PAIRS via SWDGE dma_gather (int16 pair index =
 col//2); the packed weight table ewp2 holds w at the matching parity slot
 and 0 at the other.  The segmented sum is a halving tree of contiguous
 bf16 adds (step-1 2x DVE mode) finished by a short strided fp32 reduce.
"""

import os

import numpy as np
import ml_dtypes

import concourse.bacc as bacc
import concourse.bass as bass
import concourse.mybir as mybir
import concourse.tile as tile
from concourse.masks import make_identity

P = 128
M_CORES = 8

f32 = mybir.dt.float32
bf16 = mybir.dt.bfloat16

# stash of the last run's BassKernelResults (for test harnesses)
LAST_RESULTS = None


# --------------------------------------------------------------------------
# host-side planning: integer index work only (sharding / layout)
# --------------------------------------------------------------------------
def _build_plan(row, col, N, M=M_CORES, group_tiles=3):
    E = row.size
    npc = (N + M - 1) // M              # nodes per core
    TPC = (npc + P - 1) // P            # tiles per core
    NSP = TPC * P                       # padded shard size
    ND = M * NSP

    cnt = np.bincount(row, minlength=N)

    # per-core degree-ascending order -> device positions
    gl2dev = np.empty(N, np.int64)
    for c in range(M):
        lo, hi = c * npc, min((c + 1) * npc, N)
        nodes = np.arange(lo, hi)
        ordered = nodes[np.argsort(cnt[nodes], kind="stable")]
        gl2dev[ordered] = c * NSP + np.arange(hi - lo)

    rdev = gl2dev[row]
    cdev = gl2dev[col]

    # J (max in-tile degree) per global tile, then shared per local tile
    cnt_dev = np.zeros(ND, np.int64)
    cnt_dev[gl2dev] = cnt
    J_gt = cnt_dev.reshape(M * TPC, P).max(axis=1)           # [M*TPC]
    J_t = J_gt.reshape(M, TPC).max(axis=0)                   # [TPC] shared

    # groups: full groups of `group_tiles`, last few tiles single (their max
    # degree grows fast under the degree-sort, so grouping them pads a lot)
    groups = []  # (t0, ntiles)
    tail = min(TPC, 9)
    t0 = 0
    while t0 < TPC - tail:
        nt = min(group_tiles, TPC - tail - t0)
        groups.append((t0, nt))
        t0 += nt
    while t0 < TPC:
        groups.append((t0, 1))
        t0 += 1

    g_meta = []  # (t0, nt, JG, off)
    off = 0
    tile2g = np.empty(TPC, np.int64)
    for gi, (t0, nt) in enumerate(groups):
        JG = int(max(1, J_t[t0 : t0 + nt].max()))
        g_meta.append((t0, nt, JG, off))
        tile2g[t0 : t0 + nt] = gi
        off += nt * JG
    S = off

    # slot position of each edge
    order = np.argsort(rdev, kind="stable")
    rs = rdev[order]
    # occurrence index within row
    first = np.r_[True, rs[1:] != rs[:-1]]
    idx_of_first = np.flatnonzero(first)
    grp_start = np.repeat(idx_of_first, np.diff(np.r_[idx_of_first, rs.size]))
    j = np.arange(rs.size) - grp_start

    ce = rs // NSP
    loc = rs % NSP
    t_loc = loc // P
    p = loc % P
    g = tile2g[t_loc]
    g_t0 = np.array([m[0] for m in g_meta])[g]
    g_JG = np.array([m[2] for m in g_meta])[g]
    g_off = np.array([m[3] for m in g_meta])[g]
    pos = g_off + (t_loc - g_t0) * g_JG + j
    assert pos.max() < S

    return dict(
        N=N, E=E, M=M, npc=npc, TPC=TPC, NSP=NSP, ND=ND, S=S,
        groups=g_meta, gl2dev=gl2dev,
        scatter=(ce, p, pos, order), cdev=cdev,
    )


def _pack_inputs(plan, x, edge_weight, row, col):
    """Pack edge tables for the paired-node dma_gather scheme.

    The normalized Laplacian weight w = -dis[row]*ew*dis[col] is computed on
    the host and baked into ewp2: the slot's pair holds w at parity
    col_dev%2 and 0.0 at the other, so the mul+reduce selects the right
    node of the gathered 256B pair.
    """
    M, Pn, S = plan["M"], P, plan["S"]
    ND, NSP, C = plan["ND"], plan["NSP"], x.shape[1]
    N = plan["N"]
    ce, p, pos, order = plan["scatter"]
    cdev = plan["cdev"][order].astype(np.int64)

    # host-side symmetric normalization (the reference formula)
    deg = np.bincount(row, weights=edge_weight.astype(np.float64),
                      minlength=N).astype(np.float32)
    dis = np.where(deg > 0, 1.0 / np.sqrt(deg.astype(np.float32)), 0.0
                   ).astype(np.float32)
    w = (-dis[row] * edge_weight.astype(np.float32) * dis[col])[order]

    ewp2_full = np.zeros((M, Pn, S, 2), np.float32)
    ewp2_full[ce, p, pos, cdev % 2] = w

    pair_full = np.zeros((M, Pn, S), np.int16)
    pair_full[ce, p, pos] = (cdev // 2).astype(np.int16)

    # int16 index tables for dma_gather, per core / per group:
    # flat slot i = chunk*128 + p ; table[pp, s] = flat[s*16 + pp%16]
    idx16_full = np.zeros((M, Pn, 8 * S), np.int16)
    for g in (plan["groups"]):
        t0, nt, JG, off = g
        ns = nt * JG
        for c in range(M):
            flat = pair_full[c][:, off:off + ns].T.reshape(-1)  # i = s*128+p
            tab = flat.reshape(8 * ns, 16).T  # [16, 8*ns]
            idx16_full[c][:, 8 * off: 8 * (off + ns)] = np.tile(tab, (8, 1))

    x_dev = np.zeros((ND, C), np.float32)
    x_dev[plan["gl2dev"]] = x.astype(np.float32)
    x_rep = x_dev.astype(ml_dtypes.bfloat16)

    ewp2_bf = ewp2_full.reshape(M, Pn, 2 * S).astype(ml_dtypes.bfloat16)
    return idx16_full, ewp2_bf, x_dev, x_rep


# --------------------------------------------------------------------------
# device program
# --------------------------------------------------------------------------
def build_nc(plan, C, K):
    M, TPC, NSP, ND, S = plan["M"], plan["TPC"], plan["NSP"], plan["ND"], plan["S"]
    groups = plan["groups"]

    nc = bacc.Bacc("TRN2", target_bir_lowering=False, debug=False,
                   num_devices=M, num_swdge_queues=4,
                   dynamic_dma_scratch_size=32768)

    x_loc_t = nc.dram_tensor("x_loc", [NSP, C], f32, kind="ExternalInput")
    x_rep_t = nc.dram_tensor("x_rep", [ND, C], bf16, kind="ExternalInput")
    ewp2_t = nc.dram_tensor("ewp2", [P, 2 * S], bf16, kind="ExternalInput")
    idx16_t = nc.dram_tensor("idx16", [P, 8 * S], mybir.dt.int16,
                             kind="ExternalInput")
    w_t = nc.dram_tensor("W", [K, C, C], f32, kind="ExternalInput")
    b_t = nc.dram_tensor("b", [1, C], f32, kind="ExternalInput")
    out_t = nc.dram_tensor("out", [NSP, C], f32, kind="ExternalOutput")

    rg = [list(range(M))]

    with tile.TileContext(nc) as tc:
        with (
            tc.tile_pool(name="const", bufs=1) as constp,
            tc.tile_pool(name="resident", bufs=1) as resp,
            tc.tile_pool(name="gath", bufs=5) as gathp,
            tc.tile_pool(name="small", bufs=4) as smallp,
            tc.tile_pool(name="psum_t", bufs=2, space="PSUM") as psumt,
            tc.tile_pool(name="psum_o", bufs=2, space="PSUM") as psumo,
            tc.tile_pool(name="dram", bufs=1, space="DRAM") as dramp,
        ):
            # ---------------- constants ----------------
            ident = constp.tile([P, P], f32)
            make_identity(nc, ident[:])
            ones_row = constp.tile([1, P], f32)
            nc.vector.memset(ones_row[:], 1.0)
            b_sb = constp.tile([1, C], f32)
            nc.sync.dma_start(out=b_sb[:], in_=b_t[:])
            w_sb = constp.tile([C, K * C], f32)
            for k in range(K):
                nc.sync.dma_start(out=w_sb[:, k * C:(k + 1) * C], in_=w_t[k])

            # resident tables
            ewp2_sb = resp.tile([P, 2 * S], bf16)
            nc.sync.dma_start(out=ewp2_sb[:], in_=ewp2_t[:])
            idx16_sb = resp.tile([P, 8 * S], mybir.dt.int16)
            nc.sync.dma_start(out=idx16_sb[:], in_=idx16_t[:])
            x_sb = resp.tile([P, TPC * C], f32)
            nc.sync.dma_start(
                out=x_sb[:].rearrange("p (t c) -> p t c", t=TPC),
                in_=x_loc_t[:].rearrange("(t p) c -> p t c", p=P))

            t1_all = resp.tile([P, TPC * C], f32)
            cc_in = dramp.tile([NSP, C], bf16)
            cc_out = dramp.tile([ND, C], bf16, addr_space="Shared")

            # ---------------- one propagation pass ----------------
            # gathers 256B bf16 node-PAIRS via dma_gather (int16 pair
            # indices); single_packet=True emits one 256B packet per
            # descriptor; one call is capped at 1024 indices -> chunk to
            # NS_MAX=8 slots and round-robin the 4 SWDGE queues.
            NS_MAX = int(os.environ.get("KERNEL_NS_MAX", "8"))
            qctr = [0]

            def prop(src_dram, target_cb, post_cb):
                src_pairs = src_dram[:].rearrange("(q two) c -> q (two c)",
                                                  two=2)
                for (t0, nt, JG, off) in groups:
                    ns = nt * JG
                    g_tile = gathp.tile([P, ns * 2 * C], bf16, tag="gath")
                    for c0 in range(0, ns, NS_MAX):
                        cs = min(NS_MAX, ns - c0)
                        num = 128 * cs
                        nc.gpsimd.dma_gather(
                            out_ap=g_tile[:, c0 * 2 * C:(c0 + cs) * 2 * C]
                            .rearrange("p (s c) -> p s c", s=cs),
                            in_ap=src_pairs,
                            idxs_ap=idx16_sb[:, 8 * (off + c0):
                                             8 * (off + c0 + cs)],
                            num_idxs=num,
                            num_idxs_reg=num,
                            elem_size=2 * C,
                            single_packet=True,
                            queue_num=qctr[0] % 4,
                        )
                        qctr[0] += 1
                    nc.vector.tensor_tensor(
                        out=g_tile[:].rearrange("p (s c) -> p s c", s=2 * ns),
                        in0=ewp2_sb[:, 2 * off: 2 * (off + ns)].to_broadcast(
                            [P, 2 * ns, C]),
                        in1=g_tile[:].rearrange("p (s c) -> p s c", s=2 * ns),
                        op=mybir.AluOpType.mult)
                    # segmented sum over the 2*JG subslots of each tile-node:
                    # halving tree of contiguous bf16 adds (step-1 -> 2x DVE
                    # mode) down to <=4 subslots, then one short strided
                    # tensor_reduce with fp32 accumulate.
                    def seg(j0, jn):
                        return g_tile[:].rearrange(
                            "p (t j c) -> p t j c", t=nt, j=2 * JG
                        )[:, :, j0:j0 + jn, :]
                    Jcur = 2 * JG
                    while Jcur > 4:
                        half = Jcur // 2
                        rem = Jcur - half
                        nc.vector.tensor_tensor(
                            out=seg(0, half), in0=seg(0, half),
                            in1=seg(rem, half),
                            op=mybir.AluOpType.add)
                        Jcur = rem
                    nc.vector.tensor_reduce(
                        out=target_cb(t0, nt),
                        in_=g_tile[:].rearrange(
                            "p (t j c) -> p t c j", t=nt, j=2 * JG
                        )[:, :, :, 0:Jcur],
                        axis=mybir.AxisListType.X,
                        op=mybir.AluOpType.add)
                    post_cb(t0, nt)

            # ---- pass 1: T1 straight into t1_all; bf16 copy -> cc_in ----
            def p1_target(t0, nt):
                return t1_all[:, t0 * C:(t0 + nt) * C]

            def p1_post(t0, nt):
                t1bf = smallp.tile([P, nt * C], bf16, tag="t1bf")
                nc.scalar.activation(
                    out=t1bf[:], in_=t1_all[:, t0 * C:(t0 + nt) * C],
                    func=mybir.ActivationFunctionType.Copy)
                nc.sync.dma_start(
                    out=cc_in[t0 * P:(t0 + nt) * P, :].rearrange(
                        "(t p) c -> p t c", p=P),
                    in_=t1bf[:].rearrange("p (t c) -> p t c", t=nt))

            prop(x_rep_t, p1_target, p1_post)

            # ---- AllGather T1 ----
            nc.gpsimd.collective_compute(
                "AllGather", mybir.AluOpType.bypass,
                replica_groups=rg,
                ins=[cc_in[:]], outs=[cc_out[:]],
            )

            # ---- pass 2 + combine ----
            # out = relu(T0@(W0-W2) + T1@W1 + P2@(2*W2) + b); the W folding
            # happened on the host, so srcs are just (x, T1, P2).
            red2_box = {}

            def p2_target(t0, nt):
                red2 = smallp.tile([P, nt * C], f32, tag="red2")
                red2_box[t0] = red2
                return red2[:]

            def p2_post(t0, nt):
                red2 = red2_box.pop(t0)
                for ti in range(nt):
                    t = t0 + ti
                    po = psumo.tile([P, C], f32, space="PSUM")
                    srcs = (
                        x_sb[:, t * C:(t + 1) * C],
                        t1_all[:, t * C:(t + 1) * C],
                        red2[:, ti * C:(ti + 1) * C],
                    )
                    tkTs = []
                    for k in range(3):
                        pt = psumt.tile([C, P], f32, space="PSUM")
                        nc.tensor.transpose(
                            out=pt[:], in_=srcs[k], identity=ident[:])
                        tkT = smallp.tile([C, P], f32, tag=f"tkT{k}")
                        nc.scalar.activation(
                            out=tkT[:], in_=pt[:],
                            func=mybir.ActivationFunctionType.Copy)
                        tkTs.append(tkT)
                    for k in range(3):
                        nc.tensor.matmul(
                            out=po[:], lhsT=tkTs[k][:],
                            rhs=w_sb[:, k * C:(k + 1) * C],
                            start=(k == 0), stop=False)
                    nc.tensor.matmul(
                        out=po[:], lhsT=ones_row[:], rhs=b_sb[:],
                        start=False, stop=True)
                    ot = smallp.tile([P, C], f32, tag="ot")
                    nc.scalar.activation(
                        out=ot[:], in_=po[:],
                        func=mybir.ActivationFunctionType.Relu)
                    nc.sync.dma_start(
                        out=out_t[t * P:(t + 1) * P, :], in_=ot[:])

            prop(cc_out, p2_target, p2_post)

    nc.compile()
    return nc


# --------------------------------------------------------------------------
# entry point
# --------------------------------------------------------------------------
def _prepare(x, edge_index, edge_weight):
    x = np.ascontiguousarray(np.asarray(x, dtype=np.float32))
    edge_index = np.asarray(edge_index)
    edge_weight = np.ascontiguousarray(np.asarray(edge_weight, np.float32))
    row = np.asarray(edge_index[0], np.int64)
    col = np.asarray(edge_index[1], np.int64)
    N = x.shape[0]

    plan = _build_plan(row, col, N, M_CORES)
    packed = _pack_inputs(plan, x, edge_weight, row, col)
    return (plan,) + packed


def _make_in_maps(plan, idx16_full, ewp2_full, x_dev, x_rep, W, b):
    M, NSP = plan["M"], plan["NSP"]
    C = x_dev.shape[1]
    # fold T2 = 2*P2 - T0 into the weight matrices
    Wf = np.ascontiguousarray(
        np.stack([W[0] - W[2], W[1], 2.0 * W[2]]).astype(np.float32))
    in_maps = []
    for c in range(M):
        in_maps.append({
            "x_loc": np.ascontiguousarray(x_dev[c * NSP:(c + 1) * NSP]),
            "x_rep": x_rep,
            "ewp2": np.ascontiguousarray(ewp2_full[c]),
            "idx16": np.ascontiguousarray(idx16_full[c]),
            "W": Wf,
            "b": b.reshape(1, C),
        })
    return in_maps


def kernel(x, edge_index, edge_weight, W, b):
    global LAST_RESULTS
    W = np.ascontiguousarray(np.asarray(W, np.float32))
    b = np.ascontiguousarray(np.asarray(b, np.float32))
    N, C = np.asarray(x).shape
    K = W.shape[0]
    M = M_CORES

    plan, idx16_full, ewp2_full, x_dev, x_rep = _prepare(
        x, edge_index, edge_weight)
    nc = build_nc(plan, C, K)
    in_maps = _make_in_maps(plan, idx16_full, ewp2_full, x_dev, x_rep, W, b)

    trace = False
    if os.environ.get("KERNEL_TRACE") == "1":
        try:
            import antenv.axon_hooks  # noqa: F401  (injected by test harness)
            trace = True
        except ImportError:
            pass

    from concourse.bass_utils import run_bass_kernel_spmd
    res = run_bass_kernel_spmd(nc, in_maps, core_ids=list(range(M)),
                               trace=trace)
    LAST_RESULTS = res

    big = np.concatenate([r["out"] for r in res.results], axis=0)
    return big[plan["gl2dev"]]


# revision 16
# speedup vs baseline: 1.1495x; 1.1495x over previous
"""ChebConv (K=3) GNN message-passing kernel for 8 Trainium2 NeuronCores.

Strategy (node sharding, per sharding hint):
 - 50000 nodes split into 8 contiguous shards of 6250, padded to 6272 = 49*128.
 - Within each shard nodes are ordered by (integer) degree so each 128-row
   tile has near-uniform max degree -> low padding in the slot layout.
 - Edge (r, c) lives on the core owning r, at slot (tile(r), part(r), j).
 - The symmetric normalization is folded on the HOST into the per-edge
   weights:  w_e = -dis[row]*ew_e*dis[col]  (L_hat = -D^-1/2 A D^-1/2), so
   the device never computes deg/dis, and pass 1 gathers the replicated raw
   x table directly (no first AllGather):
     T1   = segsum_j(w * gather(x_rep, col))        per local (tile,node)
     T1  -> AllGather -> T1_full (bf16)
     P2   = segsum_j(w * gather(T1_full, col))
     out  = relu(T0@(W0-W2) + T1@W1 + P2@(2*W2) + b)
   (T2 = 2*P2 - T0 is folded into the weight matrices on the host.)

 Gathers move 256B bf16 node PAIRS via SWDGE dma_gather (int16 pair index =
 col//2); the packed weight table ewp2 holds w at the matching parity slot
 and 0 at the other.  The segmented sum is a halving tree of contiguous
 bf16 adds (step-1 2x DVE mode) finished by a short strided fp32 reduce.
"""

import os

import numpy as np
import ml_dtypes

import concourse.bacc as bacc
import concourse.bass as bass
import concourse.mybir as mybir
import concourse.tile as tile
from concourse.masks import make_identity

P = 128
M_CORES = 8

f32 = mybir.dt.float32
bf16 = mybir.dt.bfloat16

# stash of the last run's BassKernelResults (for test harnesses)
LAST_RESULTS = None


# --------------------------------------------------------------------------
# host-side planning: integer index work only (sharding / layout)
# --------------------------------------------------------------------------
def _build_plan(row, col, N, M=M_CORES, group_tiles=2):
    E = row.size
    npc = (N + M - 1) // M              # nodes per core
    TPC = (npc + P - 1) // P            # tiles per core
    NSP = TPC * P                       # padded shard size
    ND = M * NSP

    cnt = np.bincount(row, minlength=N)

    # per-core degree-ascending order -> device positions
    gl2dev = np.empty(N, np.int64)
    for c in range(M):
        lo, hi = c * npc, min((c + 1) * npc, N)
        nodes = np.arange(lo, hi)
        ordered = nodes[np.argsort(cnt[nodes], kind="stable")]
        gl2dev[ordered] = c * NSP + np.arange(hi - lo)

    rdev = gl2dev[row]
    cdev = gl2dev[col]

    # J (max in-tile degree) per global tile, then shared per local tile
    cnt_dev = np.zeros(ND, np.int64)
    cnt_dev[gl2dev] = cnt
    J_gt = cnt_dev.reshape(M * TPC, P).max(axis=1)           # [M*TPC]
    J_t = J_gt.reshape(M, TPC).max(axis=0)                   # [TPC] shared

    # groups: full groups of `group_tiles`, last few tiles single (their max
    # degree grows fast under the degree-sort, so grouping them pads a lot)
    groups = []  # (t0, ntiles)
    tail = min(TPC, 12)
    t0 = 0
    while t0 < TPC - tail:
        nt = min(group_tiles, TPC - tail - t0)
        groups.append((t0, nt))
        t0 += nt
    while t0 < TPC:
        groups.append((t0, 1))
        t0 += 1

    g_meta = []  # (t0, nt, JG, off)
    off = 0
    tile2g = np.empty(TPC, np.int64)
    for gi, (t0, nt) in enumerate(groups):
        JG = int(max(1, J_t[t0 : t0 + nt].max()))
        g_meta.append((t0, nt, JG, off))
        tile2g[t0 : t0 + nt] = gi
        off += nt * JG
    S = off

    # slot position of each edge
    order = np.argsort(rdev, kind="stable")
    rs = rdev[order]
    # occurrence index within row
    first = np.r_[True, rs[1:] != rs[:-1]]
    idx_of_first = np.flatnonzero(first)
    grp_start = np.repeat(idx_of_first, np.diff(np.r_[idx_of_first, rs.size]))
    j = np.arange(rs.size) - grp_start

    ce = rs // NSP
    loc = rs % NSP
    t_loc = loc // P
    p = loc % P
    g = tile2g[t_loc]
    g_t0 = np.array([m[0] for m in g_meta])[g]
    g_JG = np.array([m[2] for m in g_meta])[g]
    g_off = np.array([m[3] for m in g_meta])[g]
    pos = g_off + (t_loc - g_t0) * g_JG + j
    assert pos.max() < S

    return dict(
        N=N, E=E, M=M, npc=npc, TPC=TPC, NSP=NSP, ND=ND, S=S,
        groups=g_meta, gl2dev=gl2dev,
        scatter=(ce, p, pos, order), cdev=cdev,
    )


def _pack_inputs(plan, x, edge_weight, row, col):
    """Pack edge tables for the paired-node dma_gather scheme.

    The normalized Laplacian weight w = -dis[row]*ew*dis[col] is computed on
    the host and baked into ewp2: the slot's pair holds w at parity
    col_dev%2 and 0.0 at the other, so the mul+reduce selects the right
    node of the gathered 256B pair.
    """
    M, Pn, S = plan["M"], P, plan["S"]
    ND, NSP, C = plan["ND"], plan["NSP"], x.shape[1]
    N = plan["N"]
    ce, p, pos, order = plan["scatter"]
    cdev = plan["cdev"][order].astype(np.int64)

    # host-side symmetric normalization (the reference formula)
    deg = np.bincount(row, weights=edge_weight.astype(np.float64),
                      minlength=N).astype(np.float32)
    dis = np.where(deg > 0, 1.0 / np.sqrt(deg.astype(np.float32)), 0.0
                   ).astype(np.float32)
    w = (-dis[row] * edge_weight.astype(np.float32) * dis[col])[order]

    ewp2_full = np.zeros((M, Pn, S, 2), np.float32)
    ewp2_full[ce, p, pos, cdev % 2] = w

    pair_full = np.zeros((M, Pn, S), np.int16)
    pair_full[ce, p, pos] = (cdev // 2).astype(np.int16)

    # int16 index tables for dma_gather, per core / per group:
    # flat slot i = chunk*128 + p ; table[pp, s] = flat[s*16 + pp%16]
    idx16_full = np.zeros((M, Pn, 8 * S), np.int16)
    for g in (plan["groups"]):
        t0, nt, JG, off = g
        ns = nt * JG
        for c in range(M):
            flat = pair_full[c][:, off:off + ns].T.reshape(-1)  # i = s*128+p
            tab = flat.reshape(8 * ns, 16).T  # [16, 8*ns]
            idx16_full[c][:, 8 * off: 8 * (off + ns)] = np.tile(tab, (8, 1))

    x_dev = np.zeros((ND, C), np.float32)
    x_dev[plan["gl2dev"]] = x.astype(np.float32)
    x_rep = x_dev.astype(ml_dtypes.bfloat16)

    ewp2_bf = ewp2_full.reshape(M, Pn, 2 * S).astype(ml_dtypes.bfloat16)
    return idx16_full, ewp2_bf, x_dev, x_rep


# --------------------------------------------------------------------------
# device program
# --------------------------------------------------------------------------
def build_nc(plan, C, K):
    M, TPC, NSP, ND, S = plan["M"], plan["TPC"], plan["NSP"], plan["ND"], plan["S"]
    groups = plan["groups"]

    nc = bacc.Bacc("TRN2", target_bir_lowering=False, debug=False,
                   num_devices=M, num_swdge_queues=4,
                   dynamic_dma_scratch_size=32768)

    x_loc_t = nc.dram_tensor("x_loc", [NSP, C], f32, kind="ExternalInput")
    x_rep_t = nc.dram_tensor("x_rep", [ND, C], bf16, kind="ExternalInput")
    ewp2_t = nc.dram_tensor("ewp2", [P, 2 * S], bf16, kind="ExternalInput")
    idx16_t = nc.dram_tensor("idx16", [P, 8 * S], mybir.dt.int16,
                             kind="ExternalInput")
    w_t = nc.dram_tensor("W", [K, C, C], f32, kind="ExternalInput")
    b_t = nc.dram_tensor("b", [1, C], f32, kind="ExternalInput")
    out_t = nc.dram_tensor("out", [NSP, C], f32, kind="ExternalOutput")

    rg = [list(range(M))]

    with tile.TileContext(nc) as tc:
        with (
            tc.tile_pool(name="const", bufs=1) as constp,
            tc.tile_pool(name="resident", bufs=1) as resp,
            tc.tile_pool(name="gath", bufs=4) as gathp,
            tc.tile_pool(name="small", bufs=3) as smallp,
            tc.tile_pool(name="psum_t", bufs=2, space="PSUM") as psumt,
            tc.tile_pool(name="psum_o", bufs=2, space="PSUM") as psumo,
            tc.tile_pool(name="dram", bufs=1, space="DRAM") as dramp,
        ):
            # ---------------- constants ----------------
            ident = constp.tile([P, P], f32)
            make_identity(nc, ident[:])
            ones_row = constp.tile([1, P], f32)
            nc.vector.memset(ones_row[:], 1.0)
            b_sb = constp.tile([1, C], f32)
            nc.sync.dma_start(out=b_sb[:], in_=b_t[:])
            w_sb = constp.tile([C, K * C], f32)
            for k in range(K):
                nc.sync.dma_start(out=w_sb[:, k * C:(k + 1) * C], in_=w_t[k])

            # resident tables
            ewp2_sb = resp.tile([P, 2 * S], bf16)
            nc.sync.dma_start(out=ewp2_sb[:], in_=ewp2_t[:])
            idx16_sb = resp.tile([P, 8 * S], mybir.dt.int16)
            nc.sync.dma_start(out=idx16_sb[:], in_=idx16_t[:])
            x_sb = resp.tile([P, TPC * C], f32)
            nc.sync.dma_start(
                out=x_sb[:].rearrange("p (t c) -> p t c", t=TPC),
                in_=x_loc_t[:].rearrange("(t p) c -> p t c", p=P))

            t1_all = resp.tile([P, TPC * C], f32)
            cc_in = dramp.tile([NSP, C], bf16)
            cc_out = dramp.tile([ND, C], bf16, addr_space="Shared")

            # ---------------- one propagation pass ----------------
            # gathers 256B bf16 node-PAIRS via dma_gather (int16 pair
            # indices); single_packet=True emits one 256B packet per
            # descriptor; one call is capped at 1024 indices -> chunk to
            # NS_MAX=8 slots and round-robin the 4 SWDGE queues.
            NS_MAX = int(os.environ.get("KERNEL_NS_MAX", "8"))
            qctr = [0]

            def prop(src_dram, target_cb, post_cb):
                src_pairs = src_dram[:].rearrange("(q two) c -> q (two c)",
                                                  two=2)
                for (t0, nt, JG, off) in groups:
                    ns = nt * JG
                    g_tile = gathp.tile([P, ns * 2 * C], bf16, tag="gath")
                    for c0 in range(0, ns, NS_MAX):
                        cs = min(NS_MAX, ns - c0)
                        num = 128 * cs
                        nc.gpsimd.dma_gather(
                            out_ap=g_tile[:, c0 * 2 * C:(c0 + cs) * 2 * C]
                            .rearrange("p (s c) -> p s c", s=cs),
                            in_ap=src_pairs,
                            idxs_ap=idx16_sb[:, 8 * (off + c0):
                                             8 * (off + c0 + cs)],
                            num_idxs=num,
                            num_idxs_reg=num,
                            elem_size=2 * C,
                            single_packet=True,
                            queue_num=qctr[0] % 4,
                        )
                        qctr[0] += 1
                    nc.vector.tensor_tensor(
                        out=g_tile[:].rearrange("p (s c) -> p s c", s=2 * ns),
                        in0=ewp2_sb[:, 2 * off: 2 * (off + ns)].to_broadcast(
                            [P, 2 * ns, C]),
                        in1=g_tile[:].rearrange("p (s c) -> p s c", s=2 * ns),
                        op=mybir.AluOpType.mult)
                    # segmented sum over the 2*JG subslots of each tile-node:
                    # halving tree of contiguous bf16 adds (step-1 -> 2x DVE
                    # mode) down to <=4 subslots, then one short strided
                    # tensor_reduce with fp32 accumulate.
                    def seg(j0, jn):
                        return g_tile[:].rearrange(
                            "p (t j c) -> p t j c", t=nt, j=2 * JG
                        )[:, :, j0:j0 + jn, :]
                    Jcur = 2 * JG
                    while Jcur > 4:
                        half = Jcur // 2
                        rem = Jcur - half
                        nc.vector.tensor_tensor(
                            out=seg(0, half), in0=seg(0, half),
                            in1=seg(rem, half),
                            op=mybir.AluOpType.add)
                        Jcur = rem
                    nc.vector.tensor_reduce(
                        out=target_cb(t0, nt),
                        in_=g_tile[:].rearrange(
                            "p (t j c) -> p t c j", t=nt, j=2 * JG
                        )[:, :, :, 0:Jcur],
                        axis=mybir.AxisListType.X,
                        op=mybir.AluOpType.add)
                    post_cb(t0, nt)

            # ---- pass 1: T1 straight into t1_all; bf16 copy -> cc_in ----
            def p1_target(t0, nt):
                return t1_all[:, t0 * C:(t0 + nt) * C]

            def p1_post(t0, nt):
                t1bf = smallp.tile([P, nt * C], bf16, tag="t1bf")
                nc.scalar.activation(
                    out=t1bf[:], in_=t1_all[:, t0 * C:(t0 + nt) * C],
                    func=mybir.ActivationFunctionType.Copy)
                nc.sync.dma_start(
                    out=cc_in[t0 * P:(t0 + nt) * P, :].rearrange(
                        "(t p) c -> p t c", p=P),
                    in_=t1bf[:].rearrange("p (t c) -> p t c", t=nt))

            prop(x_rep_t, p1_target, p1_post)

            # ---- AllGather T1 ----
            nc.gpsimd.collective_compute(
                "AllGather", mybir.AluOpType.bypass,
                replica_groups=rg,
                ins=[cc_in[:]], outs=[cc_out[:]],
            )

            # ---- pass 2 + combine ----
            # out = relu(T0@(W0-W2) + T1@W1 + P2@(2*W2) + b); the W folding
            # happened on the host, so srcs are just (x, T1, P2).
            red2_box = {}

            def p2_target(t0, nt):
                red2 = smallp.tile([P, nt * C], f32, tag="red2")
                red2_box[t0] = red2
                return red2[:]

            def p2_post(t0, nt):
                red2 = red2_box.pop(t0)
                for ti in range(nt):
                    t = t0 + ti
                    po = psumo.tile([P, C], f32, space="PSUM")
                    srcs = (
                        x_sb[:, t * C:(t + 1) * C],
                        t1_all[:, t * C:(t + 1) * C],
                        red2[:, ti * C:(ti + 1) * C],
                    )
                    tkTs = []
                    for k in range(3):
                        pt = psumt.tile([C, P], f32, space="PSUM")
                        nc.tensor.transpose(
                            out=pt[:], in_=srcs[k], identity=ident[:])
                        tkT = smallp.tile([C, P], f32, tag=f"tkT{k}")
                        nc.scalar.activation(
                            out=tkT[:], in_=pt[:],
                            func=mybir.ActivationFunctionType.Copy)
                        tkTs.append(tkT)
                    for k in range(3):
                        nc.tensor.matmul(
                            out=po[:], lhsT=tkTs[k][:],
                            rhs=w_sb[:, k * C:(k + 1) * C],
                            start=(k == 0), stop=False)
                    nc.tensor.matmul(
                        out=po[:], lhsT=ones_row[:], rhs=b_sb[:],
                        start=False, stop=True)
                    ot = smallp.tile([P, C], f32, tag="ot")
                    nc.scalar.activation(
                        out=ot[:], in_=po[:],
                        func=mybir.ActivationFunctionType.Relu)
                    nc.sync.dma_start(
                        out=out_t[t * P:(t + 1) * P, :], in_=ot[:])

            prop(cc_out, p2_target, p2_post)

    nc.compile()
    return nc


# --------------------------------------------------------------------------
# entry point
# --------------------------------------------------------------------------
def _prepare(x, edge_index, edge_weight):
    x = np.ascontiguousarray(np.asarray(x, dtype=np.float32))
    edge_index = np.asarray(edge_index)
    edge_weight = np.ascontiguousarray(np.asarray(edge_weight, np.float32))
    row = np.asarray(edge_index[0], np.int64)
    col = np.asarray(edge_index[1], np.int64)
    N = x.shape[0]

    plan = _build_plan(row, col, N, M_CORES)
    packed = _pack_inputs(plan, x, edge_weight, row, col)
    return (plan,) + packed


def _make_in_maps(plan, idx16_full, ewp2_full, x_dev, x_rep, W, b):
    M, NSP = plan["M"], plan["NSP"]
    C = x_dev.shape[1]
    # fold T2 = 2*P2 - T0 into the weight matrices
    Wf = np.ascontiguousarray(
        np.stack([W[0] - W[2], W[1], 2.0 * W[2]]).astype(np.float32))
    in_maps = []
    for c in range(M):
        in_maps.append({
            "x_loc": np.ascontiguousarray(x_dev[c * NSP:(c + 1) * NSP]),
            "x_rep": x_rep,
            "ewp2": np.ascontiguousarray(ewp2_full[c]),
            "idx16": np.ascontiguousarray(idx16_full[c]),
            "W": Wf,
            "b": b.reshape(1, C),
        })
    return in_maps


def kernel(x, edge_index, edge_weight, W, b):
    global LAST_RESULTS
    W = np.ascontiguousarray(np.asarray(W, np.float32))
    b = np.ascontiguousarray(np.asarray(b, np.float32))
    N, C = np.asarray(x).shape
    K = W.shape[0]
    M = M_CORES

    plan, idx16_full, ewp2_full, x_dev, x_rep = _prepare(
        x, edge_index, edge_weight)
    nc = build_nc(plan, C, K)
    in_maps = _make_in_maps(plan, idx16_full, ewp2_full, x_dev, x_rep, W, b)

    trace = False
    if os.environ.get("KERNEL_TRACE") == "1":
        try:
            import antenv.axon_hooks  # noqa: F401  (injected by test harness)
            trace = True
        except ImportError:
            pass

    from concourse.bass_utils import run_bass_kernel_spmd
    res = run_bass_kernel_spmd(nc, in_maps, core_ids=list(range(M)),
                               trace=trace)
    LAST_RESULTS = res

    big = np.concatenate([r["out"] for r in res.results], axis=0)
    return big[plan["gl2dev"]]
